# revision 7
# baseline (speedup 1.0000x reference)
"""MoE gate routing kernel for Trainium2 (8 NeuronCores).

Strategy
--------
Tokens (8192) are sharded across 8 cores (1024 each). The [256, 7168] gate
weight is replicated. All layout work (transpose to [h, tok], fp16 hi/lo
splitting) happens on the host so the device does only matmuls + the top-k
selection chain.

Precision: x*16 and W*1024 are each split into an fp16 hi + fp16 lo pair
(exact residual split). logits = (xh@Wh + xl@Wh + xh@Wl) / 16384 gives
fp32-class logits (validated: identical top-k decisions to an exact fp32
matmul on the real problem data) while the PE runs at 1 cycle/row instead
of fp32's 4.

Per 128-token tile the device computes logits in PSUM ([128, 512]: columns
0:256 accumulate xh@Wh + xl@Wh, columns 256:512 accumulate xh@Wl), then:
sigmoid (ScalarE), +bias, group top-2 via segmented reduce_max +
match_replace, top-4 group mask, masked top-8 via the DVE Max8/MaxIndex8
ops, uncorrected-score gather via GpSimd indirect_copy, and normalization.
"""

import sys

for _p in ("/opt/trn_rl_repo", "/opt/pypackages"):
    if _p not in sys.path:
        sys.path.insert(0, _p)

import numpy as np

N_CORES = 8
T = 8192
H = 7168
E = 256
TOPK = 8
N_GROUP = 8
EPG = E // N_GROUP  # 32 experts per group
TILES_PER_CORE = 8  # 8 x 128 = 1024 tokens per core
NTILES = N_CORES * TILES_PER_CORE
HC = H // 128  # 56 contraction chunks
X_SCALE = 16.0
W_SCALE = 1024.0
INV_SCALE = 1.0 / (X_SCALE * W_SCALE)
NEG_BIG = -1.0e30

_cache = {}


def _build_bass(repeat=1, hw_loop=1):
    import concourse.bacc as bacc
    import concourse.tile as tile
    import concourse.mybir as mybir

    f16 = mybir.dt.float16
    f32 = mybir.dt.float32
    u16 = mybir.dt.uint16
    i32 = mybir.dt.int32

    nc = bacc.Bacc("TRN2", target_bir_lowering=False, debug=False,
                   num_devices=N_CORES)

    xh_d = nc.dram_tensor("xh", [TILES_PER_CORE, 128, H], f16,
                          kind="ExternalInput")
    xl_d = nc.dram_tensor("xl", [TILES_PER_CORE, 128, H], f16,
                          kind="ExternalInput")
    w_d = nc.dram_tensor("wcat", [128, HC, 2 * E], f16, kind="ExternalInput")
    b_d = nc.dram_tensor("biasrep", [128, E], f32, kind="ExternalInput")
    oi_d = nc.dram_tensor("oidx", [TILES_PER_CORE, 128, TOPK], i32,
                          kind="ExternalOutput")
    ow_d = nc.dram_tensor("ow", [TILES_PER_CORE, 128, TOPK], f32,
                          kind="ExternalOutput")

    with tile.TileContext(nc) as tc:
        with tc.tile_pool(name="wpool", bufs=1) as wpool, \
             tc.tile_pool(name="xpool", bufs=2) as xpool, \
             tc.tile_pool(name="pspool", bufs=3, space="PSUM") as pspool, \
             tc.tile_pool(name="spool", bufs=2) as spool, \
             tc.tile_pool(name="kpool", bufs=2) as kpool:

            w_sb = wpool.tile([128, HC, 2 * E], f16)
            nc.sync.dma_start(w_sb[:], w_d[:])
            bias_sb = wpool.tile([128, E], f32)
            nc.sync.dma_start(bias_sb[:], b_d[:])

            import contextlib
            loop_ctx = (tc.For_i(0, hw_loop, 1) if hw_loop > 1
                        else contextlib.nullcontext())
            with loop_ctx:
              for t in [t for _ in range(repeat)
                        for t in range(TILES_PER_CORE)]:
                xh_t = xpool.tile([128, HC, 128], f16, tag="xh")
                nc.sync.dma_start(xh_t[:], xh_d[t].rearrange(
                    "p (c k) -> p c k", c=HC))
                xl_t = xpool.tile([128, HC, 128], f16, tag="xl")
                nc.sync.dma_start(xl_t[:], xl_d[t].rearrange(
                    "p (c k) -> p c k", c=HC))

                ps = pspool.tile([128, 2 * E], f32)
                for c in range(HC):
                    last = c == HC - 1
                    # xh @ [Wh | Wl]  -> full 512 cols
                    nc.tensor.matmul(ps[:], xh_t[:, c, :], w_sb[:, c, :],
                                     start=(c == 0), stop=False)
                    # xl @ Wh -> cols 0:256
                    nc.tensor.matmul(ps[:, 0:E], xl_t[:, c, :],
                                     w_sb[:, c, 0:E],
                                     start=False, stop=last)

                # logits = (ps[:, :256] + ps[:, 256:]) * INV_SCALE (in sigmoid)
                rawb = spool.tile([128, E], f32, tag="rawb")
                nc.scalar.activation(rawb[:], ps[:, E:2 * E],
                                     mybir.ActivationFunctionType.Copy)
                raw = spool.tile([128, E], f32, tag="raw")
                nc.vector.tensor_add(raw[:], ps[:, 0:E], rawb[:])
                scores = spool.tile([128, E], f32, tag="scores")
                nc.scalar.activation(scores[:], raw[:],
                                     mybir.ActivationFunctionType.Sigmoid,
                                     bias=0.0, scale=INV_SCALE)
                s4c = spool.tile([128, E], f32, tag="s4c")
                nc.vector.tensor_add(s4c[:], scores[:], bias_sb[:])

                # group top-2 sums
                s4c_g = s4c[:].rearrange("p (g k) -> p g k", g=N_GROUP)
                m1 = kpool.tile([128, N_GROUP], f32, tag="m1")
                nc.vector.tensor_reduce(m1[:], s4c_g, mybir.AxisListType.X,
                                        mybir.AluOpType.max)
                s4m = spool.tile([128, E], f32, tag="s4m")
                nc.vector.match_replace(s4m[:], m1[:], s4c[:], NEG_BIG)
                m2 = kpool.tile([128, N_GROUP], f32, tag="m2")
                nc.vector.tensor_reduce(m2[:],
                                        s4m[:].rearrange("p (g k) -> p g k",
                                                         g=N_GROUP),
                                        mybir.AxisListType.X,
                                        mybir.AluOpType.max)
                gsc = kpool.tile([128, N_GROUP], f32, tag="gsc")
                nc.vector.tensor_add(gsc[:], m1[:], m2[:])

                # top-4 groups -> additive penalty per group
                gs8 = kpool.tile([128, 8], f32, tag="gs8")
                nc.vector.max(gs8[:], gsc[:])
                pen = kpool.tile([128, N_GROUP], f32, tag="pen")
                # pen = (gsc < t4) * NEG_BIG   (0 for selected groups)
                nc.vector.tensor_scalar(pen[:], gsc[:], gs8[:, 3:4], NEG_BIG,
                                        mybir.AluOpType.is_lt,
                                        mybir.AluOpType.mult)

                # masked scores = s4c + pen (broadcast over 32)
                masked = spool.tile([128, E], f32, tag="masked")
                nc.vector.tensor_tensor(
                    masked[:].rearrange("p (g k) -> p g k", g=N_GROUP),
                    s4c_g,
                    pen[:, :, None].broadcast_to((128, N_GROUP, EPG)),
                    mybir.AluOpType.add)

                # top-8 values + indices (corrected-score order = output order)
                v8 = kpool.tile([128, 8], f32, tag="v8")
                nc.vector.max(v8[:], masked[:])
                i16 = kpool.tile([128, 8], u16, tag="i16")
                nc.vector.max_index(i16[:], v8[:], masked[:])
                iout = kpool.tile([128, 8], i32, tag="iout")
                nc.vector.tensor_copy(iout[:], i16[:])

                # one-hot of the selected positions: knock the 8 winners out
                # of `masked`, then compare
                m2r = spool.tile([128, E], f32, tag="m2r")
                nc.vector.match_replace(m2r[:], v8[:], masked[:], NEG_BIG)
                oneh = spool.tile([128, E], f32, tag="oneh")
                nc.vector.tensor_tensor(oneh[:], masked[:], m2r[:],
                                        mybir.AluOpType.not_equal)
                pen2 = spool.tile([128, E], f32, tag="pen2")
                nc.vector.tensor_scalar(pen2[:], oneh[:], 1.0, -NEG_BIG,
                                        mybir.AluOpType.subtract,
                                        mybir.AluOpType.mult)
                um = spool.tile([128, E], f32, tag="um")
                nc.vector.tensor_add(um[:], scores[:], pen2[:])

                # selected UNCORRECTED scores (value order) + their indices
                v8u = kpool.tile([128, 8], f32, tag="v8u")
                nc.vector.max(v8u[:], um[:])
                i8u = kpool.tile([128, 8], u16, tag="i8u")
                nc.vector.max_index(i8u[:], v8u[:], um[:])

                # normalize: w = v8u / sum(v8u) * 2.5 (still value order)
                den = kpool.tile([128, 1], f32, tag="den")
                nc.vector.tensor_reduce(den[:], v8u[:],
                                        mybir.AxisListType.X,
                                        mybir.AluOpType.add)
                rec = kpool.tile([128, 1], f32, tag="rec")
                nc.vector.reciprocal(rec[:], den[:])
                v8n = kpool.tile([128, 8], f32, tag="v8n")
                nc.vector.tensor_scalar(v8n[:], v8u[:], rec[:, 0:1], 2.5,
                                        mybir.AluOpType.mult,
                                        mybir.AluOpType.mult)

                # re-pair into corrected order: w8[j] = sum_k
                #   (i8u[k] == i16[j]) * v8n[k]
                idxf = kpool.tile([128, 8], f32, tag="idxf")
                nc.vector.tensor_copy(idxf[:], i16[:])
                i8uf = kpool.tile([128, 8], f32, tag="i8uf")
                nc.vector.tensor_copy(i8uf[:], i8u[:])
                eq = kpool.tile([128, 8, 8], f32, tag="eq")
                nc.vector.tensor_tensor(
                    eq[:],
                    i8uf[:, None, :].broadcast_to((128, 8, 8)),
                    idxf[:, :, None].broadcast_to((128, 8, 8)),
                    mybir.AluOpType.is_equal)
                wm = kpool.tile([128, 8, 8], f32, tag="wm")
                nc.vector.tensor_tensor(
                    wm[:], eq[:],
                    v8n[:, None, :].broadcast_to((128, 8, 8)),
                    mybir.AluOpType.mult)
                w8 = kpool.tile([128, 8], f32, tag="w8")
                nc.vector.tensor_reduce(w8[:], wm[:],
                                        mybir.AxisListType.X,
                                        mybir.AluOpType.add)

                nc.sync.dma_start(oi_d[t], iout[:])
                nc.sync.dma_start(ow_d[t], w8[:])

    nc.compile()
    return nc


def _host_prep(hidden_states, weight, e_score_correction_bias):
    x = np.ascontiguousarray(hidden_states.reshape(T, H), dtype=np.float32)
    xs = x * np.float32(X_SCALE)
    xh = xs.astype(np.float16)
    xl = (xs - xh.astype(np.float32)).astype(np.float16)

    # [T, H] -> [ntile, 128tok, H] -> transpose h into partitions:
    # slab[tile][p, c, j] = x[tile*128 + j, c*128 + p]
    def pack_x(a):
        a4 = a.reshape(NTILES, 128, HC, 128)        # [tile, tok, c, p]
        return np.ascontiguousarray(a4.transpose(0, 3, 2, 1))  # [tile,p,c,tok]

    xh_p = pack_x(xh).reshape(NTILES, 128, H)
    xl_p = pack_x(xl).reshape(NTILES, 128, H)

    ws = weight.astype(np.float32) * np.float32(W_SCALE)
    wh = ws.astype(np.float16)
    wl = (ws - wh.astype(np.float32)).astype(np.float16)
    wcat = np.empty((128, HC, 2 * E), dtype=np.float16)
    wcat[:, :, 0:E] = wh.reshape(E, HC, 128).transpose(2, 1, 0)
    wcat[:, :, E:2 * E] = wl.reshape(E, HC, 128).transpose(2, 1, 0)
    wcat = np.ascontiguousarray(wcat)

    bias_rep = np.ascontiguousarray(
        np.broadcast_to(e_score_correction_bias.astype(np.float32)[None, :],
                        (128, E)))
    return xh_p, xl_p, wcat, bias_rep


def kernel(hidden_states, weight, e_score_correction_bias,
           _run_opts=None):
    from concourse.bass_utils import run_bass_kernel_spmd

    xh_p, xl_p, wcat, bias_rep = _host_prep(
        np.asarray(hidden_states), np.asarray(weight),
        np.asarray(e_score_correction_bias))

    if "nc" not in _cache:
        _cache["nc"] = _build_bass()
    nc = _cache["nc"]

    in_maps = []
    for core in range(N_CORES):
        sl = slice(core * TILES_PER_CORE, (core + 1) * TILES_PER_CORE)
        in_maps.append({
            "xh": xh_p[sl],
            "xl": xl_p[sl],
            "wcat": wcat,
            "biasrep": bias_rep,
        })

    opts = _run_opts or {}
    res = run_bass_kernel_spmd(nc, in_maps, core_ids=list(range(N_CORES)),
                               **opts)
    idx = np.concatenate([r["oidx"].reshape(-1, TOPK) for r in res.results])
    w = np.concatenate([r["ow"].reshape(-1, TOPK) for r in res.results])
    if _run_opts is not None:
        _cache["last_results"] = res
    return idx.astype(np.int32), w.astype(np.float32)


# revision 13
# speedup vs baseline: 4.3816x; 4.3816x over previous
"""MoE gate routing kernel for Trainium2 (8 NeuronCores).

Strategy
--------
Tokens (8192) are sharded across 8 cores (1024 each). The [256, 7168] gate
weight is replicated. All layout work (transpose to [h, tok], fp16 hi/lo
splitting) happens on the host so the device does only matmuls + the top-k
selection chain.

Precision: x*16 and W*1024 are each split into an fp16 hi + fp16 lo pair
(exact residual split). logits = (xh@Wh + xl@Wh + xh@Wl) / 16384 gives
fp32-class logits (validated: identical top-k decisions to an exact fp32
matmul on the real problem data) while the PE runs at 1 cycle/row instead
of fp32's 4.

Per 128-token tile the device computes logits in PSUM ([128, 512]: columns
0:256 accumulate xh@Wh + xl@Wh, columns 256:512 accumulate xh@Wl), then:
sigmoid (ScalarE), +bias, group top-2 via segmented reduce_max +
match_replace, top-4 group mask, masked top-8 via the DVE Max8/MaxIndex8
ops, uncorrected-score gather via GpSimd indirect_copy, and normalization.
"""

import sys

for _p in ("/opt/trn_rl_repo", "/opt/pypackages"):
    if _p not in sys.path:
        sys.path.insert(0, _p)

import numpy as np

N_CORES = 8
T = 8192
H = 7168
E = 256
TOPK = 8
N_GROUP = 8
EPG = E // N_GROUP  # 32 experts per group
TILES_PER_CORE = 8  # 8 x 128 = 1024 tokens per core
NTILES = N_CORES * TILES_PER_CORE
HC = H // 128  # 56 contraction chunks
X_SCALE = 16.0
W_SCALE = 1024.0
INV_SCALE = 1.0 / (X_SCALE * W_SCALE)
NEG_BIG = -1.0e30

_cache = {}


def _build_bass(repeat=1, hw_loop=1, variant="full"):
    import concourse.bacc as bacc
    import concourse.tile as tile
    import concourse.mybir as mybir

    f16 = mybir.dt.float16
    f32 = mybir.dt.float32
    u16 = mybir.dt.uint16
    i32 = mybir.dt.int32

    nc = bacc.Bacc("TRN2", target_bir_lowering=False, debug=False,
                   num_devices=N_CORES)

    xh_d = nc.dram_tensor("xh", [TILES_PER_CORE, 128, H], f16,
                          kind="ExternalInput")
    xl_d = nc.dram_tensor("xl", [TILES_PER_CORE, 128, H], f16,
                          kind="ExternalInput")
    w_d = nc.dram_tensor("wcat", [128, HC, 2 * E], f16, kind="ExternalInput")
    b_d = nc.dram_tensor("biasrep", [128, E], f32, kind="ExternalInput")
    oi_d = nc.dram_tensor("oidx", [TILES_PER_CORE, 128, TOPK], i32,
                          kind="ExternalOutput")
    ow_d = nc.dram_tensor("ow", [TILES_PER_CORE, 128, TOPK], f32,
                          kind="ExternalOutput")

    with tile.TileContext(nc) as tc:
        with tc.tile_pool(name="wpool", bufs=1) as wpool, \
             tc.tile_pool(name="xpool", bufs=2) as xpool, \
             tc.tile_pool(name="pspool", bufs=3, space="PSUM") as pspool, \
             tc.tile_pool(name="spool", bufs=2) as spool, \
             tc.tile_pool(name="kpool", bufs=2) as kpool:

            # W arrives in 4 chunk-groups so tile-0 matmuls can start after
            # the first ~1.8MB instead of the full 7.3MB
            WG = 4
            WGC = HC // WG  # 14 chunks per group
            w_gs = []
            for g in range(WG):
                wg = wpool.tile([128, WGC, 2 * E], f16, tag=f"wg{g}")
                w_gs.append(wg)
            nc.sync.dma_start(w_gs[0][:], w_d[:, 0:WGC, :])
            bias_sb = wpool.tile([128, E], f32)
            if hw_loop > 1:
                # benching variant: keep W loads out of the hardware loop
                for g in range(1, WG):
                    nc.sync.dma_start(w_gs[g][:],
                                      w_d[:, g * WGC:(g + 1) * WGC, :])
            nc.sync.dma_start(bias_sb[:], b_d[:])

            def w_chunk(c, lo=0, hi=2 * E):
                return w_gs[c // WGC][:, c % WGC, lo:hi]

            HHC = HC // 2  # 28: chunks per half-slab

            def load_x(t):
                """Load tile t's x as 4 half-slabs (hi/lo x first/second)."""
                slabs = []
                for d_t, tag in ((xh_d, "xh"), (xl_d, "xl")):
                    for h in range(2):
                        s = xpool.tile([128, HHC, 128], f16,
                                       tag=f"{tag}{h}")
                        nc.sync.dma_start(
                            s[:],
                            d_t[t, :, h * HHC * 128:(h + 1) * HHC * 128]
                            .rearrange("p (c k) -> p c k", c=HHC))
                        slabs.append(s)
                return slabs  # [xh_a, xh_b, xl_a, xl_b]

            def x_chunk(slabs, hi, c):
                base = 0 if hi else 2
                return slabs[base + c // HHC][:, c % HHC, :]

            tiles_seq = [t for _ in range(repeat)
                         for t in range(TILES_PER_CORE)]
            import contextlib
            loop_ctx = (tc.For_i(0, hw_loop, 1) if hw_loop > 1
                        else contextlib.nullcontext())
            with loop_ctx:
              x_slabs = {0: load_x(tiles_seq[0])}
              if hw_loop == 1:
                  for g in range(1, WG):
                      nc.sync.dma_start(w_gs[g][:],
                                        w_d[:, g * WGC:(g + 1) * WGC, :])
              for ti, t in enumerate(tiles_seq):
                if ti + 1 < len(tiles_seq):
                    x_slabs[ti + 1] = load_x(tiles_seq[ti + 1])
                slabs = x_slabs.pop(ti)

                ps = pspool.tile([128, 2 * E], f32)
                if variant != "dma_only":
                    for c in range(HC):
                        last = c == HC - 1
                        # xh @ [Wh | Wl]  -> full 512 cols
                        nc.tensor.matmul(ps[:], x_chunk(slabs, True, c),
                                         w_chunk(c),
                                         start=(c == 0), stop=False)
                        # xl @ Wh -> cols 0:256
                        nc.tensor.matmul(ps[:, 0:E], x_chunk(slabs, False, c),
                                         w_chunk(c, 0, E),
                                         start=False, stop=last)
                if variant in ("dma_only", "mm_only"):
                    dummy_i = kpool.tile([128, 8], i32, tag="iout")
                    nc.vector.tensor_copy(dummy_i[:],
                                          bias_sb[:, 0:8].bitcast(i32))
                    dummy_w = kpool.tile([128, 8], f32, tag="w8")
                    nc.vector.tensor_copy(dummy_w[:], bias_sb[:, 0:8])
                    nc.sync.dma_start(oi_d[t], dummy_i[:])
                    nc.sync.dma_start(ow_d[t], dummy_w[:])
                    continue

                # logits = (ps[:, :256] + ps[:, 256:]) * INV_SCALE (in sigmoid)
                rawb = spool.tile([128, E], f32, tag="rawb")
                nc.scalar.activation(rawb[:], ps[:, E:2 * E],
                                     mybir.ActivationFunctionType.Copy)
                raw = spool.tile([128, E], f32, tag="raw")
                nc.vector.tensor_add(raw[:], ps[:, 0:E], rawb[:])
                scores = spool.tile([128, E], f32, tag="scores")
                nc.scalar.activation(scores[:], raw[:],
                                     mybir.ActivationFunctionType.Sigmoid,
                                     bias=0.0, scale=INV_SCALE)
                s4c = spool.tile([128, E], f32, tag="s4c")
                nc.vector.tensor_add(s4c[:], scores[:], bias_sb[:])

                # group top-2 sums
                s4c_g = s4c[:].rearrange("p (g k) -> p g k", g=N_GROUP)
                m1 = kpool.tile([128, N_GROUP], f32, tag="m1")
                nc.vector.tensor_reduce(m1[:], s4c_g, mybir.AxisListType.X,
                                        mybir.AluOpType.max)
                s4m = spool.tile([128, E], f32, tag="s4m")
                nc.vector.match_replace(s4m[:], m1[:], s4c[:], NEG_BIG)
                m2 = kpool.tile([128, N_GROUP], f32, tag="m2")
                nc.vector.tensor_reduce(m2[:],
                                        s4m[:].rearrange("p (g k) -> p g k",
                                                         g=N_GROUP),
                                        mybir.AxisListType.X,
                                        mybir.AluOpType.max)
                gsc = kpool.tile([128, N_GROUP], f32, tag="gsc")
                nc.vector.tensor_add(gsc[:], m1[:], m2[:])

                # top-4 groups -> additive penalty per group
                gs8 = kpool.tile([128, 8], f32, tag="gs8")
                nc.vector.max(gs8[:], gsc[:])
                pen = kpool.tile([128, N_GROUP], f32, tag="pen")
                # pen = (gsc < t4) * NEG_BIG   (0 for selected groups)
                nc.vector.tensor_scalar(pen[:], gsc[:], gs8[:, 3:4], NEG_BIG,
                                        mybir.AluOpType.is_lt,
                                        mybir.AluOpType.mult)

                # masked scores = s4c + pen (broadcast over 32)
                masked = spool.tile([128, E], f32, tag="masked")
                nc.vector.tensor_tensor(
                    masked[:].rearrange("p (g k) -> p g k", g=N_GROUP),
                    s4c_g,
                    pen[:, :, None].broadcast_to((128, N_GROUP, EPG)),
                    mybir.AluOpType.add)

                # top-8 values + indices (corrected-score order = output order)
                v8 = kpool.tile([128, 8], f32, tag="v8")
                nc.vector.max(v8[:], masked[:])
                i16 = kpool.tile([128, 8], u16, tag="i16")
                nc.vector.max_index(i16[:], v8[:], masked[:])
                iout = kpool.tile([128, 8], i32, tag="iout")
                nc.vector.tensor_copy(iout[:], i16[:])

                # one-hot of the selected positions: knock the 8 winners out
                # of `masked`, then compare
                m2r = spool.tile([128, E], f32, tag="m2r")
                nc.vector.match_replace(m2r[:], v8[:], masked[:], NEG_BIG)
                oneh = spool.tile([128, E], f32, tag="oneh")
                nc.vector.tensor_tensor(oneh[:], masked[:], m2r[:],
                                        mybir.AluOpType.not_equal)
                pen2 = spool.tile([128, E], f32, tag="pen2")
                nc.vector.tensor_scalar(pen2[:], oneh[:], 1.0, -NEG_BIG,
                                        mybir.AluOpType.subtract,
                                        mybir.AluOpType.mult)
                um = spool.tile([128, E], f32, tag="um")
                nc.vector.tensor_add(um[:], scores[:], pen2[:])

                # selected UNCORRECTED scores (value order) + their indices
                v8u = kpool.tile([128, 8], f32, tag="v8u")
                nc.vector.max(v8u[:], um[:])
                i8u = kpool.tile([128, 8], u16, tag="i8u")
                nc.vector.max_index(i8u[:], v8u[:], um[:])

                # normalize: w = v8u / sum(v8u) * 2.5 (still value order)
                den = kpool.tile([128, 1], f32, tag="den")
                nc.vector.tensor_reduce(den[:], v8u[:],
                                        mybir.AxisListType.X,
                                        mybir.AluOpType.add)
                rec = kpool.tile([128, 1], f32, tag="rec")
                nc.vector.reciprocal(rec[:], den[:])
                v8n = kpool.tile([128, 8], f32, tag="v8n")
                nc.vector.tensor_scalar(v8n[:], v8u[:], rec[:, 0:1], 2.5,
                                        mybir.AluOpType.mult,
                                        mybir.AluOpType.mult)

                # re-pair into corrected order: w8[j] = sum_k
                #   (i8u[k] == i16[j]) * v8n[k]
                idxf = kpool.tile([128, 8], f32, tag="idxf")
                nc.vector.tensor_copy(idxf[:], i16[:])
                i8uf = kpool.tile([128, 8], f32, tag="i8uf")
                nc.vector.tensor_copy(i8uf[:], i8u[:])
                eq = kpool.tile([128, 8, 8], f32, tag="eq")
                nc.vector.tensor_tensor(
                    eq[:],
                    i8uf[:, None, :].broadcast_to((128, 8, 8)),
                    idxf[:, :, None].broadcast_to((128, 8, 8)),
                    mybir.AluOpType.is_equal)
                wm = kpool.tile([128, 8, 8], f32, tag="wm")
                nc.vector.tensor_tensor(
                    wm[:], eq[:],
                    v8n[:, None, :].broadcast_to((128, 8, 8)),
                    mybir.AluOpType.mult)
                w8 = kpool.tile([128, 8], f32, tag="w8")
                nc.vector.tensor_reduce(w8[:], wm[:],
                                        mybir.AxisListType.X,
                                        mybir.AluOpType.add)

                nc.sync.dma_start(oi_d[t], iout[:])
                nc.sync.dma_start(ow_d[t], w8[:])

    nc.compile()
    return nc


def _host_prep(hidden_states, weight, e_score_correction_bias):
    x = np.ascontiguousarray(hidden_states.reshape(T, H), dtype=np.float32)
    xs = x * np.float32(X_SCALE)
    xh = xs.astype(np.float16)
    xl = (xs - xh.astype(np.float32)).astype(np.float16)

    # [T, H] -> [ntile, 128tok, H] -> transpose h into partitions:
    # slab[tile][p, c, j] = x[tile*128 + j, c*128 + p]
    def pack_x(a):
        a4 = a.reshape(NTILES, 128, HC, 128)        # [tile, tok, c, p]
        return np.ascontiguousarray(a4.transpose(0, 3, 2, 1))  # [tile,p,c,tok]

    xh_p = pack_x(xh).reshape(NTILES, 128, H)
    xl_p = pack_x(xl).reshape(NTILES, 128, H)

    ws = weight.astype(np.float32) * np.float32(W_SCALE)
    wh = ws.astype(np.float16)
    wl = (ws - wh.astype(np.float32)).astype(np.float16)
    wcat = np.empty((128, HC, 2 * E), dtype=np.float16)
    wcat[:, :, 0:E] = wh.reshape(E, HC, 128).transpose(2, 1, 0)
    wcat[:, :, E:2 * E] = wl.reshape(E, HC, 128).transpose(2, 1, 0)
    wcat = np.ascontiguousarray(wcat)

    bias_rep = np.ascontiguousarray(
        np.broadcast_to(e_score_correction_bias.astype(np.float32)[None, :],
                        (128, E)))
    return xh_p, xl_p, wcat, bias_rep


def kernel(hidden_states, weight, e_score_correction_bias,
           _run_opts=None):
    from concourse.bass_utils import run_bass_kernel_spmd

    xh_p, xl_p, wcat, bias_rep = _host_prep(
        np.asarray(hidden_states), np.asarray(weight),
        np.asarray(e_score_correction_bias))

    if "nc" not in _cache:
        _cache["nc"] = _build_bass()
    nc = _cache["nc"]

    in_maps = []
    for core in range(N_CORES):
        sl = slice(core * TILES_PER_CORE, (core + 1) * TILES_PER_CORE)
        in_maps.append({
            "xh": xh_p[sl],
            "xl": xl_p[sl],
            "wcat": wcat,
            "biasrep": bias_rep,
        })

    opts = _run_opts or {}
    res = run_bass_kernel_spmd(nc, in_maps, core_ids=list(range(N_CORES)),
                               **opts)
    idx = np.concatenate([r["oidx"].reshape(-1, TOPK) for r in res.results])
    w = np.concatenate([r["ow"].reshape(-1, TOPK) for r in res.results])
    if _run_opts is not None:
        _cache["last_results"] = res
    return idx.astype(np.int32), w.astype(np.float32)
